# revision 2
# baseline (speedup 1.0000x reference)
"""DKVMN write-head memory update kernel for Trainium2 (8 NeuronCores) — v3.

Computes, for each batch row b:
    erase = sigmoid(control @ erase_W.T + erase_b)          # [B, D]
    add   = tanh(control @ add_W.T + add_b)                 # [B, D]
    new_memory[b,m,d] = memory[b,m,d] * (1 - ww[b,m]*erase[b,d]) + ww[b,m]*add[b,d]

Sharding: pure data parallel over batch B (4096 -> 512 per core), tiny Linear
weights replicated.  Host pre-transposes control_input and the Linear weights.

v4 keeps GPSIMD idle: on TRN2 the Pool engine's ONLY SBUF access is the
read+write port pair it shares with the DVE's second port, and the lock is
exclusive per instruction — so any Pool elementwise pass serializes against
all DVE 2-input work (this, not engine throughput, is what made the fp32
DVE+Pool split ~2.2x slower on hardware than the cost model predicted).
All elementwise work runs on DVE in bf16 perf modes; ACT (dedicated SBUF
ports) casts and copies; PE (dedicated ports + PSUM) does the final add as
two accumulating identity matmuls.  Per chunk:

    v    = mem * e_bc          DVE TT    fp32 in -> bf16 out
    z    = a_bc - v            DVE TT    bf16 2x (in place)
    z_m *= w_m                 DVE tensor_scalar bf16 4x (in place, per m)
    memb = bf16(mem)           ACT Copy  (dedicated port)
    psum = I@z + I@memb        PE  2 matmuls per 512-col slice, accumulated
    out  = fp32(psum)          ACT Copy  PSUM -> SBUF
    store out

Numerics: out is bf16-rounded (mem term included) -> rel err ~2e-3 vs the
2e-2 gate.  HBM traffic stays all-fp32 (the ~189us/core roofline).

DMA: loads/stores alternate between the SP and ACT HWDGE rings so both
logical DMA queues always have work (SDMA engines round-robin between queue
rows at packet granularity, hiding HBM latency); stores are emitted two
chunks late so their waits never head-of-line block a sequencer.
"""

import sys

for _p in ("/opt/trn_rl_repo",):
    if _p not in sys.path:
        sys.path.insert(0, _p)

from contextlib import ExitStack

import numpy as np

import concourse.bass as bass
import concourse.tile as tile
from concourse import mybir

N_CORES = 8
B, M, D = 4096, 128, 128
B_LOC = B // N_CORES  # 512
P = 128               # SBUF partitions = batch tile
N_BTILES = B_LOC // P  # 4

F32 = mybir.dt.float32
BF16 = mybir.dt.bfloat16
ALU = mybir.AluOpType
ACTF = mybir.ActivationFunctionType

# ---- tunables ----
CHUNK_M = 16          # m-slots per chunk -> [128, CHUNK_M*128] tiles
ACT_TS = 0            # per-chunk m-slots whose scaling runs on ACT (tail of range)
STORE_LAG = 2         # emit store of chunk i at chunk i+STORE_LAG
MM_COLS = 512         # moving-tensor free dim per matmul (PE max 512)
BUFS_MEM = 6
BUFS_WORK = 5
BUFS_CAST = 5
BUFS_OUT = 5


def legalize_waits(nc: bass.Bass) -> None:
    """Split multi-wait instructions for walrus.

    TRN2 codegen ('setupSyncWait: Too many sync wait commands') rejects
    instructions carrying more than one semaphore wait, but the Tile
    scheduler freely attaches several.  Hoist all but the last wait onto
    standalone single-wait InstEventSemaphore instructions inserted
    immediately before the instruction on the same engine."""
    for bb in nc.main_func.blocks:
        insts = bb.instructions
        if not any(
            i.sync_info is not None and i.sync_info.on_wait and len(i.sync_info.on_wait) > 1
            for i in insts
        ):
            continue
        new_list = []
        for inst in insts:
            si = inst.sync_info
            if si is not None and si.on_wait and len(si.on_wait) > 1:
                for w in si.on_wait[:-1]:
                    ev = mybir.InstEventSemaphore(
                        name=nc.get_next_instruction_name(),
                        engine=inst.engine,
                        ins=[],
                        outs=[],
                        sync_info=mybir.SyncInfo(on_wait=[w], on_update=[]),
                    )
                    nc.register_instruction(ev, overwrite=True)
                    new_list.append(ev)
                inst.sync_info = mybir.SyncInfo(
                    on_wait=[si.on_wait[-1]], on_update=list(si.on_update)
                )
            new_list.append(inst)
        bb.instructions = new_list


def build_nc(repeat: int = 1, mode: str = "full", loop: int = 0,
             chunk_m: int = CHUNK_M, act_ts: int = ACT_TS,
             store_lag: int = STORE_LAG, mm_cols: int = MM_COLS,
             preload_chunks: int = 0,
             bufs=(BUFS_MEM, BUFS_WORK, BUFS_CAST, BUFS_OUT)) -> bass.Bass:
    n_chunks = M // chunk_m
    bufs_mem, bufs_work, bufs_cast, bufs_out = bufs
    nc = bass.Bass()

    # host-transposed inputs: ctrl_t[k, b] = control[b, k]; *_w_t[k, j] = W[j, k]
    ctrlT_d = nc.dram_tensor("ctrl_t", [D, B_LOC], F32, kind="ExternalInput")
    mem_d = nc.dram_tensor("memory", [B_LOC, M, D], F32, kind="ExternalInput")
    ww_d = nc.dram_tensor("write_weight", [B_LOC, M], F32, kind="ExternalInput")
    ewT_d = nc.dram_tensor("erase_w_t", [D, D], F32, kind="ExternalInput")
    eb_d = nc.dram_tensor("erase_b", [D], F32, kind="ExternalInput")
    awT_d = nc.dram_tensor("add_w_t", [D, D], F32, kind="ExternalInput")
    ab_d = nc.dram_tensor("add_b", [D], F32, kind="ExternalInput")
    id_d = nc.dram_tensor("ident", [P, P], F32, kind="ExternalInput")
    out_d = nc.dram_tensor("new_memory", [B_LOC, M, D], F32, kind="ExternalOutput")

    with tile.TileContext(nc) as tc, ExitStack() as ctx:
        singles = ctx.enter_context(tc.tile_pool(name="singles", bufs=1))
        sig = ctx.enter_context(tc.tile_pool(name="sig", bufs=2))
        big = ctx.enter_context(tc.tile_pool(name="big", bufs=bufs_mem))
        work = ctx.enter_context(tc.tile_pool(name="work", bufs=bufs_work))
        cast = ctx.enter_context(tc.tile_pool(name="cast", bufs=bufs_cast))
        outp = ctx.enter_context(tc.tile_pool(name="outp", bufs=bufs_out))
        psum = ctx.enter_context(tc.tile_pool(name="psum", bufs=2, space="PSUM"))
        psacc = ctx.enter_context(tc.tile_pool(name="psacc", bufs=1, space="PSUM"))

        hwdge = (nc.sync, nc.scalar)

        # ---- prefetch the first chunk loads ahead of the constants so the
        # DMA rings start moving the big tensors at t=0 ----
        preloaded: dict = {}
        n_pre = min(preload_chunks, n_chunks) if mode != "dma" else 0
        for gi_pre in range(n_pre):
            m0 = gi_pre * chunk_m
            mem_t = big.tile([P, chunk_m, D], F32, tag="mem")
            hwdge[gi_pre % 2].dma_start(
                out=mem_t[:], in_=mem_d[0:P, m0 : m0 + chunk_m, :]
            )
            preloaded[gi_pre] = mem_t

        # ---- one-time constants ----
        # Linear weights, DVE-staged so the signal matmuls' waits all funnel
        # through the DVE semaphore (walrus allows only one wait on fp32 LDW).
        wt_tiles = {}
        for name, w_dram in (("e", ewT_d), ("a", awT_d)):
            w_raw = sig.tile([D, D], F32, tag="wload")
            nc.sync.dma_start(out=w_raw[:], in_=w_dram[:, :])
            w_t = singles.tile([D, D], F32, tag=f"wt_{name}")
            nc.vector.tensor_copy(w_t[:], w_raw[:])
            wt_tiles[name] = w_t

        # biases replicated across partitions via partition-broadcast DMA
        bias_bc = {}
        for name, b_dram in (("e", eb_d), ("a", ab_d)):
            b_t = singles.tile([P, D], F32, tag=f"bias_{name}")
            b_ap = bass.AP(tensor=b_dram[:].tensor, offset=0, ap=[[0, P], [1, D]])
            nc.sync.dma_start(out=b_t[:], in_=b_ap)
            bias_bc[name] = b_t

        # identity (bf16) for the PE final-add matmuls, DVE-staged
        id_raw = sig.tile([P, P], F32, tag="id_raw")
        nc.sync.dma_start(out=id_raw[:], in_=id_d[:, :])
        id_bf = singles.tile([P, P], BF16, tag="id_bf")
        nc.vector.tensor_copy(id_bf[:], id_raw[:])

        # ---- main loops (software-pipelined emission) ----
        def emit_tail(gi, v_t, mb_t, w_sb, out_t, b0, m0):
            # z_m *= w_m in place: DVE tensor_scalar (bf16 4x), optionally the
            # tail m-slots on ACT (Copy with per-partition scale).
            n_dve = chunk_m - act_ts
            for m in range(n_dve):
                nc.vector.tensor_scalar(
                    v_t[:, m, :], v_t[:, m, :], w_sb[:, m0 + m : m0 + m + 1],
                    None, ALU.mult,
                )
            for m in range(n_dve, chunk_m):
                nc.scalar.activation(
                    v_t[:, m, :], v_t[:, m, :], ACTF.Copy,
                    bias=0.0, scale=w_sb[:, m0 + m : m0 + m + 1],
                )
            # out = z*w + mem on PE: per 512-col slice, two identity matmuls
            # accumulate bf16(z*w) and bf16(mem) into PSUM; ACT copies back.
            ps = psacc.tile([P, chunk_m, D], F32, tag="acc")
            mm_m = mm_cols // D
            for sl in range(chunk_m // mm_m):
                s0 = sl * mm_m
                nc.tensor.matmul(
                    ps[:, s0 : s0 + mm_m, :], id_bf[:], v_t[:, s0 : s0 + mm_m, :],
                    start=True, stop=False,
                )
                nc.tensor.matmul(
                    ps[:, s0 : s0 + mm_m, :], id_bf[:], mb_t[:, s0 : s0 + mm_m, :],
                    start=False, stop=True,
                )
            nc.scalar.activation(out_t[:], ps[:], ACTF.Copy)

        def emit_store(gi, out_t, b0, m0):
            eng = hwdge[(gi + 1) % 2]
            eng.dma_start(out=out_d[b0 : b0 + P, m0 : m0 + chunk_m, :], in_=out_t[:])

        def emit_body():
            pend_tail = None
            pend_store = []
            gi = -1
            for _rep, bt in ((r, t) for r in range(repeat) for t in range(N_BTILES)):
                b0 = bt * P

                # ctrl^T tile for this batch tile, DVE-staged (see above)
                ctrlT_raw = sig.tile([D, P], F32, tag="ctrl_raw")
                nc.scalar.dma_start(out=ctrlT_raw[:], in_=ctrlT_d[:, b0 : b0 + P])
                ctrlT_sb = sig.tile([D, P], F32, tag="ctrl_stg")
                nc.vector.tensor_copy(ctrlT_sb[:], ctrlT_raw[:])

                # erase / add signals: psum[b, j] = sum_k ctrl[b,k] W[j,k],
                # then Pool adds the (partition-broadcast) bias, ACT applies
                # the nonlinearity.  a comes out bf16 (consumed by the bf16 z
                # pass); e stays fp32 (consumed by the fp32-input v pass).
                e_sb = sig.tile([P, D], F32, tag="e")
                a_sb = sig.tile([P, D], BF16, tag="a")
                for name, act_fn, dst in (("e", ACTF.Sigmoid, e_sb), ("a", ACTF.Tanh, a_sb)):
                    sig_ps = psum.tile([P, D], F32, tag=f"sig_{name}")
                    nc.tensor.matmul(sig_ps[:], ctrlT_sb[:], wt_tiles[name][:])
                    pre_sb = sig.tile([P, D], F32, tag=f"pre_{name}")
                    nc.vector.tensor_tensor(pre_sb[:], sig_ps[:], bias_bc[name][:], ALU.add)
                    nc.scalar.activation(dst[:], pre_sb[:], act_fn)

                w_sb = sig.tile([P, M], F32, tag="w")
                nc.scalar.dma_start(out=w_sb[:], in_=ww_d[b0 : b0 + P, :])

                for ci in range(n_chunks):
                    gi += 1
                    m0 = ci * chunk_m

                    if gi in preloaded:
                        mem_t = preloaded.pop(gi)
                    else:
                        mem_t = big.tile([P, chunk_m, D], F32, tag="mem")
                        hwdge[gi % 2].dma_start(
                            out=mem_t[:], in_=mem_d[b0 : b0 + P, m0 : m0 + chunk_m, :]
                        )

                    if mode == "dma":
                        # timing bisection: store the loaded tile straight back
                        hwdge[(gi + 1) % 2].dma_start(
                            out=out_d[b0 : b0 + P, m0 : m0 + chunk_m, :], in_=mem_t[:]
                        )
                        continue

                    e_bc = e_sb[:].unsqueeze(1).broadcast_to((P, chunk_m, D))
                    a_bc = a_sb[:].unsqueeze(1).broadcast_to((P, chunk_m, D))

                    # v = mem * erase (DVE, bf16 out); z = add - v in place
                    # (DVE TT bf16 runs in 2x mode); ACT casts mem to bf16 on
                    # its dedicated port for the PE final add.
                    v_t = work.tile([P, chunk_m, D], BF16, tag="v")
                    nc.vector.tensor_tensor(v_t[:], mem_t[:], e_bc, ALU.mult)
                    nc.vector.tensor_tensor(v_t[:], a_bc, v_t[:], ALU.subtract)
                    mb_t = cast.tile([P, chunk_m, D], BF16, tag="memb")
                    nc.scalar.activation(mb_t[:], mem_t[:], ACTF.Copy)
                    out_t = outp.tile([P, chunk_m, D], F32, tag="out")

                    if pend_tail is not None:
                        emit_tail(*pend_tail)
                        pend_store.append(
                            (pend_tail[0], pend_tail[4], pend_tail[5], pend_tail[6])
                        )
                    pend_tail = (gi, v_t, mb_t, w_sb, out_t, b0, m0)
                    while pend_store and pend_store[0][0] <= gi - store_lag:
                        emit_store(*pend_store.pop(0))

            if pend_tail is not None:
                emit_tail(*pend_tail)
                pend_store.append(
                    (pend_tail[0], pend_tail[4], pend_tail[5], pend_tail[6])
                )
            for s in pend_store:
                emit_store(*s)

        if loop:
            with tc.For_i(0, loop, 1, hint_engines=(mybir.EngineType.DVE,)):
                emit_body()
        else:
            emit_body()

    legalize_waits(nc)
    return nc


_CACHE: dict = {}


def _get_nc() -> bass.Bass:
    if "nc" not in _CACHE:
        _CACHE["nc"] = build_nc()
    return _CACHE["nc"]


def make_in_maps(**inputs) -> list:
    """Shard full inputs into per-core input maps (batch split, weights
    replicated).  control_input and the Linear weights are pre-transposed on
    the host so the device kernel needs no PE transposes."""
    ci = np.asarray(inputs["control_input"], dtype=np.float32)
    mem = np.asarray(inputs["memory"], dtype=np.float32)
    ww = np.asarray(inputs["write_weight"], dtype=np.float32)
    ewT = np.ascontiguousarray(np.asarray(inputs["erase_W"], dtype=np.float32).T)
    eb = np.ascontiguousarray(np.asarray(inputs["erase_b"], dtype=np.float32))
    awT = np.ascontiguousarray(np.asarray(inputs["add_W"], dtype=np.float32).T)
    ab = np.ascontiguousarray(np.asarray(inputs["add_b"], dtype=np.float32))
    in_maps = []
    for c in range(N_CORES):
        sl = slice(c * B_LOC, (c + 1) * B_LOC)
        in_maps.append(
            {
                "ctrl_t": np.ascontiguousarray(ci[sl].T),
                "memory": np.ascontiguousarray(mem[sl]),
                "write_weight": np.ascontiguousarray(ww[sl]),
                "erase_w_t": ewT,
                "erase_b": eb,
                "add_w_t": awT,
                "add_b": ab,
                "ident": np.eye(128, dtype=np.float32),
            }
        )
    return in_maps


def run_sharded(trace: bool = False, **inputs):
    """Run on all 8 cores; returns (full_output, BassKernelResults)."""
    from concourse.bass_utils import run_bass_kernel_spmd

    nc = _get_nc()
    res = run_bass_kernel_spmd(
        nc, make_in_maps(**inputs), core_ids=list(range(N_CORES)), trace=trace
    )
    out = np.concatenate(
        [res.results[c]["new_memory"] for c in range(N_CORES)], axis=0
    )
    return out, res


def kernel(**inputs) -> np.ndarray:
    out, _ = run_sharded(trace=False, **inputs)
    return out
